# revision 1
# baseline (speedup 1.0000x reference)
"""KNN (k=16) over B=2, N=8192, D=3 points on 8 TRN2 NeuronCores.

Strategy (v5)
-------------
Shard the 2*8192 queries across 8 cores (batch b = core//4, query chunk
core%4 of 2048 queries). Every core holds the full 8192 keys of its batch.

Bit-exact ranking key (matches the reference's roundings):
  reference: d2 = fl( fl(sqq + sqk) - fl(2*inner) ), inner = FMA-chain dot.
  Here: PE matmul with lhsT = 2*qT gives psum = fl(2*inner) EXACTLY
  (scaling every MAC input by 2 only shifts exponents, so each rounding
  is the baseline's rounding scaled by 2), ACT materializes
  nsrow = fl((-sqk) + (-sqq)) == -fl(sqq+sqk), and ACT copies psum to
  SBUF; Pool adds nd2 = fl(ch + nsrow) == -d2 bitwise.
  vs the previous baseline this removes one full-width ACT pass (the
  explicit ch = 2*psum Copy) by folding the doubling into the weights.

Top-16 per row of nd2 (descending): per-1024-chunk DVE max8 (8 calls vs
16 at 512 wide - halves the per-call overheads), then top-8 / ranks 9-16
via max8 + match_replace + max8 over the 64 candidates, and two full-row
FIND_INDEX_8 scans for the global indices (per-chunk index recovery was
measured slower: max_index calls have a large fixed cost on HW).
"""

import numpy as np

B = 2
N = 8192
K = 16
N_CORES = 8
QPC = (B * N) // N_CORES  # queries per core: 2048
QB = 128                  # query block (partition dim)
CK = 1024                 # key chunk
N_QB = QPC // QB          # 16
N_CK = N // CK            # 8
NCAND = 8 * N_CK          # 64 candidates per row
NEG_BIG = -1.0e30

_cached = {}


def _build_nc(reps=1):
    import concourse.mybir as mybir
    from concourse import bacc, tile

    f32 = mybir.dt.float32
    u32 = mybir.dt.uint32
    Copy = mybir.ActivationFunctionType.Copy
    Identity = mybir.ActivationFunctionType.Identity

    nc = bacc.Bacc()
    q2T = nc.declare_dram_parameter("q2T", [3, QPC], f32, isOutput=False)
    kT = nc.declare_dram_parameter("kT", [3, N], f32, isOutput=False)
    nsqk = nc.declare_dram_parameter("nsqk", [1, N], f32, isOutput=False)
    nsqq = nc.declare_dram_parameter("nsqq", [QB, N_QB], f32, isOutput=False)
    out = nc.declare_dram_parameter("out", [QPC, K], u32, isOutput=True)

    with tile.TileContext(nc) as tc:
        with (
            tc.tile_pool(name="const", bufs=1) as cpool,
            tc.tile_pool(name="mm", bufs=4, space="PSUM") as mmpool,
            tc.tile_pool(name="nsr", bufs=3) as nsrpool,
            tc.tile_pool(name="ch", bufs=3) as chpool,
            tc.tile_pool(name="rows", bufs=2) as rpool,
            tc.tile_pool(name="small", bufs=2) as spool,
        ):
            q2T_sb = cpool.tile([3, QPC], f32, tag="q2T", name="q2T_sb")
            nc.sync.dma_start(out=q2T_sb[:], in_=q2T[:])
            kT_sb = cpool.tile([3, N], f32, tag="kT", name="kT_sb")
            nc.sync.dma_start(out=kT_sb[:], in_=kT[:])
            nsqk_sb = cpool.tile([QB, N], f32, tag="nsqk", name="nsqk_sb")
            nc.sync.dma_start(out=nsqk_sb[:],
                              in_=nsqk[0:1, :].partition_broadcast(QB))
            nsqq_sb = cpool.tile([QB, N_QB], f32, tag="nsqq", name="nsqq_sb")
            nc.sync.dma_start(out=nsqq_sb[:], in_=nsqq[:])

            for qb in [qb for _ in range(reps) for qb in range(N_QB)]:
                row = rpool.tile([QB, N], f32, tag="row", name="row")
                V = spool.tile([QB, NCAND], f32, tag="V", name="V")

                for c in range(N_CK):
                    sl = slice(c * CK, (c + 1) * CK)
                    ps = mmpool.tile([QB, CK], f32, tag="ps", name="ps")
                    for h in range(CK // 512):
                        nc.tensor.matmul(
                            ps[:, h * 512:(h + 1) * 512],
                            lhsT=q2T_sb[:, qb * QB:(qb + 1) * QB],
                            rhs=kT_sb[:, c * CK + h * 512:
                                      c * CK + (h + 1) * 512],
                            start=True,
                            stop=True,
                        )
                    # ch = psum = fl(2*inner) (lhsT was pre-doubled)
                    ch = chpool.tile([QB, CK], f32, tag="ch", name="ch")
                    nc.scalar.activation(ch[:], ps[:], Copy, scale=1.0)
                    # nsrow = fl(-sqk + -sqq) == -fl(sqq+sqk) bitwise
                    nsr = nsrpool.tile([QB, CK], f32, tag="nsr", name="nsr")
                    nc.scalar.activation(
                        nsr[:], nsqk_sb[:, sl], Identity,
                        bias=nsqq_sb[:, qb:qb + 1], scale=1.0)
                    # row = fl(ch - fl(sqq+sqk)) = -d2 bitwise
                    nc.gpsimd.tensor_add(row[:, sl], ch[:], nsr[:])
                    nc.vector.max(V[:, c * 8:(c + 1) * 8], row[:, sl])

                a8 = spool.tile([QB, 8], f32, tag="a8", name="a8")
                b8 = spool.tile([QB, 8], f32, tag="b8", name="b8")
                Vm = spool.tile([QB, NCAND], f32, tag="Vm", name="Vm")
                ia = spool.tile([QB, 8], u32, tag="ia", name="ia")
                ib = spool.tile([QB, 8], u32, tag="ib", name="ib")

                nc.vector.max(a8[:], V[:])
                nc.vector.max_index(ia[:], a8[:], row[:])
                nc.vector.match_replace(Vm[:], a8[:], V[:], NEG_BIG)
                nc.vector.max(b8[:], Vm[:])
                nc.vector.max_index(ib[:], b8[:], row[:])

                nc.sync.dma_start(out=out[qb * QB:(qb + 1) * QB, 0:8],
                                  in_=ia[:])
                nc.sync.dma_start(out=out[qb * QB:(qb + 1) * QB, 8:16],
                                  in_=ib[:])
    nc.compile()
    return nc


def _get_nc(reps=1):
    key = f"nc{reps}"
    if key not in _cached:
        _cached[key] = _build_nc(reps)
    return _cached[key]


def _make_in_maps(points):
    pts = np.ascontiguousarray(np.asarray(points, dtype=np.float32))
    assert pts.shape == (B, N, 3), pts.shape
    # sq exactly like the reference computes it: sequential f32
    sq = ((pts[..., 0] * pts[..., 0] + pts[..., 1] * pts[..., 1])
          + pts[..., 2] * pts[..., 2]).astype(np.float32)
    in_maps = []
    for c in range(N_CORES):
        b = c // (N_CORES // B)
        qc = c % (N_CORES // B)
        q = pts[b, qc * QPC:(qc + 1) * QPC, :]
        sqq = sq[b, qc * QPC:(qc + 1) * QPC]
        in_maps.append({
            "q2T": np.ascontiguousarray((2.0 * q).T.astype(np.float32)),
            "kT": np.ascontiguousarray(pts[b].T),
            "nsqk": np.ascontiguousarray(-sq[b][None, :]),
            "nsqq": np.ascontiguousarray((-sqq).reshape(N_QB, QB).T),
        })
    return in_maps


def _make_runner(nc, n_cores):
    """Build a cached jitted SPMD executor for ``nc`` (axon PJRT path).

    Mirrors concourse.bass2jax.run_bass_via_pjrt but caches the jitted
    callable so repeated calls don't re-trace/re-compile.
    """
    import jax
    import numpy as _np
    from jax.sharding import Mesh, PartitionSpec
    try:
        from jax.experimental.shard_map import shard_map
    except ImportError:
        from jax.sharding import shard_map  # newer jax
    import concourse.mybir as mybir
    from concourse.bass2jax import (_bass_exec_p, install_neuronx_cc_hook,
                                    partition_id_tensor)

    install_neuronx_cc_hook()

    partition_name = (nc.partition_id_tensor.name
                      if nc.partition_id_tensor else None)
    in_names, out_names, out_avals, zero_outs = [], [], [], []
    for alloc in nc.m.functions[0].allocations:
        if not isinstance(alloc, mybir.MemoryLocationSet):
            continue
        name = alloc.memorylocations[0].name
        if alloc.kind == "ExternalInput":
            if name != partition_name:
                in_names.append(name)
        elif alloc.kind == "ExternalOutput":
            out_names.append(name)
            shape = tuple(alloc.tensor_shape)
            dtype = mybir.dt.np(alloc.dtype)
            out_avals.append(jax.core.ShapedArray(shape, dtype))
            zero_outs.append(_np.zeros(shape, dtype))
    n_params = len(in_names)
    n_outs = len(out_avals)
    all_in_names = list(in_names) + list(out_names)
    if partition_name is not None:
        all_in_names.append(partition_name)
    donate = tuple(range(n_params, n_params + n_outs))

    def _body(*args):
        operands = list(args)
        if partition_name is not None:
            operands.append(partition_id_tensor())
        outs = _bass_exec_p.bind(
            *operands,
            out_avals=tuple(out_avals),
            in_names=tuple(all_in_names),
            out_names=tuple(out_names),
            lowering_input_output_aliases=(),
            sim_require_finite=True,
            sim_require_nnan=True,
            nc=nc,
        )
        return tuple(outs)

    devices = jax.devices()[:n_cores]
    mesh = Mesh(np.asarray(devices), ("core",))
    in_specs = (PartitionSpec("core"),) * (n_params + n_outs)
    out_specs = (PartitionSpec("core"),) * len(out_names)
    sharded = jax.jit(
        shard_map(_body, mesh=mesh, in_specs=in_specs, out_specs=out_specs,
                  check_rep=False),
        donate_argnums=donate,
        keep_unused=True,
    )

    def execute(in_maps):
        per_core = [[np.asarray(m[nm]) for nm in in_names] for m in in_maps]
        concat_in = [
            np.concatenate([per_core[c][i] for c in range(n_cores)], axis=0)
            for i in range(n_params)
        ]
        concat_zeros = [
            np.zeros((n_cores * z.shape[0], *z.shape[1:]), z.dtype)
            for z in zero_outs
        ]
        out_arrs = sharded(*concat_in, *concat_zeros)
        out_arrs = [np.asarray(o) for o in out_arrs]
        return [
            {nm: out_arrs[i].reshape(n_cores, *out_avals[i].shape)[c]
             for i, nm in enumerate(out_names)}
            for c in range(n_cores)
        ]

    return execute


def _get_runner():
    if "runner" not in _cached:
        _cached["runner"] = _make_runner(_get_nc(), N_CORES)
    return _cached["runner"]


def _assemble(results):
    idx = np.empty((B, N, K), dtype=np.int32)
    for c in range(N_CORES):
        b = c // (N_CORES // B)
        qc = c % (N_CORES // B)
        o = np.asarray(results[c]["out"])
        idx[b, qc * QPC:(qc + 1) * QPC, :] = o.astype(np.int32)
    return idx


def run(points, k, trace=False):
    assert int(k) == K
    in_maps = _make_in_maps(points)
    last_err = None
    for attempt in range(3):
        try:
            execute = _get_runner()
            results = execute(in_maps)
            return _assemble(results), results
        except Exception as e:  # transient device wedge -> rebuild + retry
            last_err = e
            _cached.pop("runner", None)
            import time as _time
            _time.sleep(2.0 * (attempt + 1))
    raise last_err


def kernel(points, k):
    idx, _ = run(points, k)
    return idx

